# revision 27
# baseline (speedup 1.0000x reference)
"""Two-layer GraphConv (GCN) on 8 Trainium2 NeuronCores.

Reference computation (per layer):
    h   = x @ W                      [N, D]
    msg = h[edge_src] * edge_weight  [E, D]
    out = segment_sum(msg, edge_dst) [N, D]
    x'  = tanh(out)

Distribution: partition nodes across the 8 cores by destination range
(12500 nodes/core). Each core computes h for its own slice (dense matmul,
fp16), AllGathers the full h table into local DRAM, gathers h[src] rows for
its own (dst-sorted) edges with dma_gather, and aggregates per 128-dst block
with one-hot selector matmuls into PSUM, then tanh on the Scalar engine.

Perf structure (v2):
  - dma_gather descriptor generation runs on one Q7 core-pair selected by
    queue_num; issuing the 4 per-chunk gathers of a supergroup on queues
    0..3 runs them on 4 distinct pairs (~3x measured speedup). Gathers are
    one big multi-packet instruction per (supergroup, chunk).
  - idx lists are packed per queue: queue c's cpu pair reads partitions
    [32c, 32c+32), so chunk c's idx stream lives only in those rows.
  - Selector tiles are built with two wide tensor_tensor ops per supergroup
    (is_equal + mult against broadcast doff/ew columns) instead of one
    tensor_scalar per 128-slot batch.
  - Layer 1 emits its output transposed ([feat, node]) so layer 2's dense
    matmul needs no runtime transpose.

dma_gather uses int16 indices, so the h table is addressed in 4 row-chunks
(< 32768 rows each). Edges are bucketed by (dst block, src chunk); each
bucket is padded to a multiple of 128 slots with ew=0 entries so the matmul
schedule is static and identical across cores (SPMD: one instruction
stream). Schedule constants are derived from the actual edge data at call
time, then compiled fresh — correct for any input values.
"""

import numpy as np

import concourse.bacc as bacc
import concourse.mybir as mybir
import concourse.tile as tile
from concourse.bass_utils import run_bass_kernel_spmd

N_NODES = 100000
E_EDGES = 1600000
D = 128
N_CORES = 8
NPC = N_NODES // N_CORES          # 12500 nodes per core
P = 128
NBLK = (NPC + P - 1) // P         # 98 dst blocks per core (last has 84)
NCHUNK = 4
CHUNK = N_NODES // NCHUNK         # 25000-row chunks (< 32768, int16 gather idx)
G = 2                             # dst blocks per supergroup (one gather group)
NSG = NBLK // G                   # 49 supergroups
PREF = 0                          # supergroups descriptor-prefetched per layer
                                  # (prepared-DMA consumer sync proved racy on
                                  # HW — see prefetch(); 0 disables the path)

BF16 = np.float16
USE_SHARED = True


def _prep(x, edge_src, edge_dst, edge_weight):
    """Host-side scheduling: bucket edges by (core, dst-block, src-chunk),
    pad each bucket to a multiple of 128 slots (shared across cores), and
    build per-core gather-index / selector-metadata arrays."""
    src = edge_src.astype(np.int64)
    dst = edge_dst.astype(np.int64)
    ew = edge_weight.astype(np.float32)

    core = dst // NPC
    ldst = dst - core * NPC
    blk = ldst // P                       # 0..NBLK-1
    doff = (ldst % P).astype(np.float32)
    chunk = src // CHUNK
    lsrc = (src - chunk * CHUNK).astype(np.int16)

    nseg = N_CORES * NBLK * NCHUNK
    seg = (core * NBLK + blk) * NCHUNK + chunk
    counts = np.bincount(seg, minlength=nseg).reshape(N_CORES, NBLK, NCHUNK)

    # static slot count per (block, chunk): max over cores, rounded to 128
    S = ((counts.max(axis=0) + P - 1) // P) * P      # [NBLK, NCHUNK]
    Ssg = S.reshape(NSG, G, NCHUNK)
    L = Ssg.sum(axis=1)                              # [NSG, NCHUNK] gather sizes
    SLOTS_G = L.sum(axis=1)                          # [NSG] slots per supergroup

    # offsets
    REG_OFF = np.zeros((NSG, NCHUNK), dtype=np.int64)    # msg-local region start
    REG_OFF[:, 1:] = np.cumsum(L, axis=1)[:, :-1]
    # SUB_OFF[b, ch]: msg-local slot offset of block b's bucket inside its sg
    sub = np.cumsum(Ssg, axis=1)                        # cumsum over blocks in sg
    SUB_OFF = np.zeros((NSG, G, NCHUNK), dtype=np.int64)
    SUB_OFF[:, 1:, :] = sub[:, :-1, :]
    SUB_OFF = SUB_OFF + REG_OFF[:, None, :]
    # IDXC_BASE[c][g]: start of (g, c)'s idx list within chunk c's idx stream
    IDXC_BASE = np.zeros((NCHUNK, NSG), dtype=np.int64)
    IDXC_BASE[:, 1:] = np.cumsum(L.T, axis=1)[:, :-1]
    CHUNK_TOT = L.sum(axis=0)                        # [NCHUNK] idxs per chunk
    FMAX = int(((CHUNK_TOT.max() + 15) // 16 + 7) // 8 * 8)
    BAT_BASE = np.zeros(NSG, dtype=np.int64)
    BAT_BASE[1:] = np.cumsum(SLOTS_G // P)[:-1]
    NBAT = int((SLOTS_G // P).sum())

    # rank of each edge within its (core, blk, chunk) bucket
    order = np.argsort(seg, kind="stable")
    counts_flat = counts.reshape(-1)
    starts = np.zeros(nseg, dtype=np.int64)
    starts[1:] = np.cumsum(counts_flat)[:-1]
    rank_sorted = np.arange(E_EDGES, dtype=np.int64) - starts[seg[order]]
    rank = np.empty(E_EDGES, dtype=np.int64)
    rank[order] = rank_sorted

    g_of = blk // G
    # position inside the (g, chunk) gather idx list
    within = (SUB_OFF[g_of, blk % G, chunk] - REG_OFF[g_of, chunk]) + rank
    idxpos = IDXC_BASE[chunk, g_of] + within
    # msg-local slot inside the supergroup
    slot = SUB_OFF[g_of, blk % G, chunk] + rank
    bat = BAT_BASE[g_of] + slot // P
    part = slot % P

    # per-(core, chunk) idx streams, wrapped in 16 partitions, placed in the
    # 32-partition group [32c, 32c+32) that queue c's Q7 cpu pair reads
    idx_pack = np.zeros((N_CORES, P, FMAX), dtype=np.int16)
    idx_flat = np.zeros((N_CORES, NCHUNK, FMAX * 16), dtype=np.int16)
    idx_flat[core, chunk, idxpos] = lsrc
    for c in range(NCHUNK):
        w = idx_flat[:, c, :].reshape(N_CORES, FMAX, 16).transpose(0, 2, 1)
        idx_pack[:, 32 * c:32 * c + 16, :] = w
        idx_pack[:, 32 * c + 16:32 * c + 32, :] = w

    doff_arr = np.zeros((N_CORES, NBAT, P), dtype=np.float32)
    ew_arr = np.zeros((N_CORES, NBAT, P), dtype=np.float32)
    doff_arr[core, bat, part] = doff
    ew_arr[core, bat, part] = ew
    doff_cols = np.ascontiguousarray(
        doff_arr.transpose(0, 2, 1)).astype(np.float16)   # [cores, 128, NBAT]
    ew_cols = np.ascontiguousarray(
        ew_arr.transpose(0, 2, 1)).astype(np.float16)

    meta = {
        "S": S, "L": L, "SLOTS_G": SLOTS_G, "REG_OFF": REG_OFF,
        "SUB_OFF": SUB_OFF, "IDXC_BASE": IDXC_BASE, "BAT_BASE": BAT_BASE,
        "FMAX": FMAX, "NBAT": NBAT,
    }
    return meta, idx_pack, doff_cols, ew_cols


def _build(meta):
    S = meta["S"]; L = meta["L"]; SLOTS_G = meta["SLOTS_G"]
    REG_OFF = meta["REG_OFF"]; SUB_OFF = meta["SUB_OFF"]
    IDXC_BASE = meta["IDXC_BASE"]; BAT_BASE = meta["BAT_BASE"]
    FMAX = meta["FMAX"]; NBAT = meta["NBAT"]
    NPAD = NBLK * P

    nc = bacc.Bacc("TRN2", target_bir_lowering=False, debug=False,
                   num_devices=N_CORES, num_swdge_queues=4)
    xT_in = nc.dram_tensor("xT_in", [P, NPAD], mybir.dt.float16,
                           kind="ExternalInput")
    w1_in = nc.dram_tensor("w1_in", [P, D], mybir.dt.float16, kind="ExternalInput")
    w2_in = nc.dram_tensor("w2_in", [P, D], mybir.dt.float16, kind="ExternalInput")
    idx_in = nc.dram_tensor("idx_in", [P, FMAX], mybir.dt.int16,
                            kind="ExternalInput")
    doff_in = nc.dram_tensor("doff_in", [P, NBAT], mybir.dt.float16,
                             kind="ExternalInput")
    ew_in = nc.dram_tensor("ew_in", [P, NBAT], mybir.dt.float16,
                           kind="ExternalInput")
    out_dram = nc.dram_tensor("out", [NPC, D], mybir.dt.float32,
                              kind="ExternalOutput")

    with tile.TileContext(nc) as tc:
        with tc.tile_pool(name="const", bufs=1) as const, \
             tc.tile_pool(name="xt", bufs=1) as xtp, \
             tc.tile_pool(name="msg", bufs=4) as msgp, \
             tc.tile_pool(name="sel", bufs=4) as selp, \
             tc.tile_pool(name="hcast", bufs=4) as hcp, \
             tc.tile_pool(name="outp", bufs=4) as outp, \
             tc.tile_pool(name="psA", bufs=4, space="PSUM") as psA, \
             tc.tile_pool(name="psD", bufs=2, space="PSUM") as psD, \
             tc.tile_pool(name="dram", bufs=1, space="DRAM") as dram:

            # ---- resident constants ----
            iota_i32 = const.tile([P, P], mybir.dt.int32)
            nc.gpsimd.iota(iota_i32[:], pattern=[[1, P]], base=0,
                           channel_multiplier=0)
            iota_bf = const.tile([P, P], mybir.dt.float16)
            nc.vector.tensor_copy(out=iota_bf[:], in_=iota_i32[:])

            w1_t = const.tile([P, D], mybir.dt.float16)
            nc.sync.dma_start(out=w1_t[:], in_=w1_in[:])
            w2_t = const.tile([P, D], mybir.dt.float16)
            nc.sync.dma_start(out=w2_t[:], in_=w2_in[:])
            idx_t = const.tile([P, FMAX], mybir.dt.int16)
            nc.sync.dma_start(out=idx_t[:], in_=idx_in[:])
            doff_t = const.tile([P, NBAT], mybir.dt.float16)
            nc.sync.dma_start(out=doff_t[:], in_=doff_in[:])
            ew_t = const.tile([P, NBAT], mybir.dt.float16)
            nc.sync.dma_start(out=ew_t[:], in_=ew_in[:])
            x1T = xtp.tile([P, NPAD], mybir.dt.float16, tag="xT")
            nc.sync.dma_start(out=x1T[:], in_=xT_in[:])

            # ---- DRAM internals ----
            h_bounce = [dram.tile([NPC, D], mybir.dt.float16, tag=f"hb{i}",
                                  name=f"h_bounce{i}") for i in range(2)]
            h_full = [dram.tile([N_NODES, D], mybir.dt.float16, tag=f"hf{i}",
                                addr_space=("Shared" if USE_SHARED else "Local"),
                                name=f"h_full{i}")
                      for i in range(2)]

            def dense(xT_tile, w_tile, bounce):
                for t in range(NBLK):
                    ps = psD.tile([P, D], mybir.dt.float32, tag="dense")
                    nc.tensor.matmul(out=ps[:], lhsT=xT_tile[:, t * P:(t + 1) * P],
                                     rhs=w_tile[:], start=True, stop=True)
                    hb = hcp.tile([P, D], mybir.dt.float16, tag="hcast")
                    nc.scalar.activation(out=hb[:], in_=ps[:],
                                         func=mybir.ActivationFunctionType.Copy)
                    rows = min(P, NPC - t * P)
                    nc.sync.dma_start(out=bounce[t * P:t * P + rows, :],
                                      in_=hb[:rows, :])

            def allgather(bounce, full):
                nc.gpsimd.collective_compute(
                    "AllGather", mybir.AluOpType.bypass,
                    replica_groups=[list(range(N_CORES))],
                    ins=[bounce.opt()], outs=[full.opt()],
                )

            def emit_gathers(msg, full, g, sems=None):
                for ch in range(NCHUNK):
                    lg = int(L[g, ch])
                    if lg == 0:
                        continue
                    r0 = int(REG_OFF[g, ch]) // P
                    i0 = int(IDXC_BASE[ch, g]) // 16
                    kw = dict(
                        out_ap=msg[:, r0:r0 + lg // P, :],
                        in_ap=full[ch * CHUNK:(ch + 1) * CHUNK, :],
                        idxs_ap=idx_t[:, i0:i0 + lg // 16],
                        num_idxs=lg, num_idxs_reg=lg, elem_size=D,
                        queue_num=ch, single_packet=False,
                    )
                    if sems is not None:
                        nc.gpsimd.dma_gather(prepare_only=True, sem=sems[ch],
                                             **kw)
                    else:
                        nc.gpsimd.dma_gather(**kw)

            def prefetch(full, layer):
                """Emit descriptor-gen for the first PREF supergroups right
                after the AllGather: desc-gen has no data dep on the table
                (deferred to trigger_dma), so the Q7 pairs build descriptors
                while the AG is in flight and the DMAs fire the moment it
                lands. Tile's DMASW-lane bookkeeping under-synchronizes
                consumers of prepared DMAs (later in-order DMAs on the lane
                satisfy the count early), so consumers are gated manually via
                dma_sems + a Tensor-engine wait_ge in aggregate()."""
                sems = [nc.alloc_semaphore(f"prep{layer}_{ch}")
                        for ch in range(NCHUNK)]
                n_preps = [int((L[:PREF, ch] > 0).sum()) for ch in range(NCHUNK)]
                pre = {}
                for g in range(PREF):
                    nb = int(SLOTS_G[g]) // P
                    msg = msgp.tile([P, nb, D], mybir.dt.float16, tag="msg")
                    emit_gathers(msg, full, g, sems=sems)
                    pre[g] = msg
                return pre, (sems, n_preps)

            def aggregate(full, layer, x2T, pre, gate):
                sems, n_preps = gate
                if pre:
                    for ch in range(NCHUNK):
                        nc.gpsimd.trigger_dma(count=None, queue_num=ch)
                for g in range(NSG):
                    nslot = int(SLOTS_G[g])
                    nb = nslot // P
                    gb0 = int(BAT_BASE[g])
                    if g in pre:
                        msg = pre[g]
                    else:
                        msg = msgp.tile([P, nb, D], mybir.dt.float16, tag="msg")
                        emit_gathers(msg, full, g)
                    # wide selector build: sel[e, b, d] = ew[e,b]*(d==doff[e,b])
                    wsel = selp.tile([P, nb, P], mybir.dt.float16, tag="sel")
                    iota_b = iota_bf[:].unsqueeze(1).broadcast_to([P, nb, P])
                    doff_b = doff_t[:, gb0:gb0 + nb].unsqueeze(2) \
                        .broadcast_to([P, nb, P])
                    ew_b = ew_t[:, gb0:gb0 + nb].unsqueeze(2) \
                        .broadcast_to([P, nb, P])
                    if g in pre:
                        # Build the one-hot per chunk region, gating each on
                        # its queue's prepared-DMA completion semaphore; the
                        # agg matmuls read wsel so the wait propagates via
                        # real RAW deps (Tile's DMASW bookkeeping under-syncs
                        # consumers of prepared DMAs).
                        for ch in range(NCHUNK):
                            lgc = int(L[g, ch])
                            if lgc == 0:
                                continue
                            r0 = int(REG_OFF[g, ch]) // P
                            nbc = lgc // P
                            eq = nc.vector.tensor_tensor(
                                out=wsel[:, r0:r0 + nbc, :],
                                in0=iota_bf[:].unsqueeze(1)
                                    .broadcast_to([P, nbc, P]),
                                in1=doff_t[:, gb0 + r0:gb0 + r0 + nbc]
                                    .unsqueeze(2).broadcast_to([P, nbc, P]),
                                op=mybir.AluOpType.is_equal)
                            eq.wait_op(sems[ch], 16 * n_preps[ch], "sem-ge")
                    else:
                        nc.vector.tensor_tensor(out=wsel[:], in0=iota_b,
                                                in1=doff_b,
                                                op=mybir.AluOpType.is_equal)
                    nc.vector.tensor_tensor(out=wsel[:], in0=wsel[:], in1=ew_b,
                                            op=mybir.AluOpType.mult)
                    for j in range(G):
                        b = g * G + j
                        batches = []
                        for ch in range(NCHUNK):
                            nbj = int(S[b, ch]) // P
                            s0 = int(SUB_OFF[g, j, ch]) // P
                            batches += [s0 + k for k in range(nbj)]
                        rows = min(P, NPC - b * P)
                        ps = psA.tile([P, D], mybir.dt.float32, tag="agg")
                        for i, s in enumerate(batches):
                            first, last = (i == 0), (i == len(batches) - 1)
                            if layer == 1:
                                # psum[feat, dst] = sum_e msg[e, f] sel[e, d]
                                nc.tensor.matmul(out=ps[:], lhsT=msg[:, s, :],
                                                 rhs=wsel[:, s, :], start=first,
                                                 stop=last)
                            else:
                                # psum[dst, feat]
                                nc.tensor.matmul(out=ps[:], lhsT=wsel[:, s, :],
                                                 rhs=msg[:, s, :], start=first,
                                                 stop=last)
                        if layer == 1:
                            nc.scalar.activation(
                                out=x2T[:, b * P:(b + 1) * P], in_=ps[:],
                                func=mybir.ActivationFunctionType.Tanh)
                        else:
                            ot = outp.tile([P, D], mybir.dt.float32, tag="out")
                            nc.scalar.activation(
                                out=ot[:], in_=ps[:],
                                func=mybir.ActivationFunctionType.Tanh)
                            nc.sync.dma_start(out=out_dram[b * P:b * P + rows, :],
                                              in_=ot[:rows, :])

            dense(x1T, w1_t, h_bounce[0])
            allgather(h_bounce[0], h_full[0])
            pre1, gate1 = prefetch(h_full[0], 1)
            x2T = xtp.tile([P, NPAD], mybir.dt.float16, tag="xT")
            aggregate(h_full[0], 1, x2T, pre1, gate1)
            dense(x2T, w2_t, h_bounce[1])
            allgather(h_bounce[1], h_full[1])
            pre2, gate2 = prefetch(h_full[1], 2)
            aggregate(h_full[1], 2, None, pre2, gate2)

    nc.compile()
    return nc


def kernel(x, edge_src, edge_dst, edge_weight, W1, W2, _trace=False):
    assert x.shape == (N_NODES, D) and edge_src.shape == (E_EDGES,)
    meta, idx_pack, doff_c, ew_c = _prep(x, edge_src, edge_dst, edge_weight)
    nc = _build(meta)

    NPAD = NBLK * P
    w1 = np.ascontiguousarray(W1.astype(BF16))
    w2 = np.ascontiguousarray(W2.astype(BF16))
    in_maps = []
    for c in range(N_CORES):
        xT = np.zeros((P, NPAD), dtype=BF16)
        xT[:, :NPC] = x[c * NPC:(c + 1) * NPC].T.astype(BF16)
        in_maps.append({
            "xT_in": xT,
            "w1_in": w1, "w2_in": w2,
            "idx_in": idx_pack[c],
            "doff_in": doff_c[c],
            "ew_in": ew_c[c],
        })
    res = run_bass_kernel_spmd(nc, in_maps, core_ids=list(range(N_CORES)),
                               trace=_trace)
    out = np.concatenate([res.results[c]["out"] for c in range(N_CORES)], axis=0)
    if _trace:
        kernel.last_results = res
    return out


# revision 29
# speedup vs baseline: 1.0360x; 1.0360x over previous
"""Two-layer GraphConv (GCN) on 8 Trainium2 NeuronCores.

Reference computation (per layer):
    h   = x @ W                      [N, D]
    msg = h[edge_src] * edge_weight  [E, D]
    out = segment_sum(msg, edge_dst) [N, D]
    x'  = tanh(out)

Distribution: partition nodes across the 8 cores by destination range
(12500 nodes/core). Each core computes h for its own slice (dense matmul,
fp16), AllGathers the full h table into local DRAM, gathers h[src] rows for
its own (dst-sorted) edges with dma_gather, and aggregates per 128-dst block
with one-hot selector matmuls into PSUM, then tanh on the Scalar engine.

Perf structure (v2):
  - dma_gather descriptor generation runs on one Q7 core-pair selected by
    queue_num; issuing the 4 per-chunk gathers of a supergroup on queues
    0..3 runs them on 4 distinct pairs (~3x measured speedup). Gathers are
    one big multi-packet instruction per (supergroup, chunk).
  - idx lists are packed per queue: queue c's cpu pair reads partitions
    [32c, 32c+32), so chunk c's idx stream lives only in those rows.
  - Selector tiles are built with two wide tensor_tensor ops per supergroup
    (is_equal + mult against broadcast doff/ew columns) instead of one
    tensor_scalar per 128-slot batch.
  - Layer 1 emits its output transposed ([feat, node]) so layer 2's dense
    matmul needs no runtime transpose.

dma_gather uses int16 indices, so the h table is addressed in 4 row-chunks
(< 32768 rows each). Edges are bucketed by (dst block, src chunk); each
bucket is padded to a multiple of 128 slots with ew=0 entries so the matmul
schedule is static and identical across cores (SPMD: one instruction
stream). Schedule constants are derived from the actual edge data at call
time, then compiled fresh — correct for any input values.
"""

import numpy as np

import concourse.bacc as bacc
import concourse.mybir as mybir
import concourse.tile as tile
from concourse.bass_utils import run_bass_kernel_spmd

N_NODES = 100000
E_EDGES = 1600000
D = 128
N_CORES = 8
NPC = N_NODES // N_CORES          # 12500 nodes per core
P = 128
NBLK = (NPC + P - 1) // P         # 98 dst blocks per core (last has 84)
NCHUNK = 4
CHUNK = N_NODES // NCHUNK         # 25000-row chunks (< 32768, int16 gather idx)
G = 2                             # dst blocks per supergroup (one gather group)
NSG = NBLK // G                   # 49 supergroups
PREF = 0                          # supergroups descriptor-prefetched per layer
                                  # (prepared-DMA consumer sync proved racy on
                                  # HW — see prefetch(); 0 disables the path)

BF16 = np.float16
USE_SHARED = True


def _prep(x, edge_src, edge_dst, edge_weight):
    """Host-side scheduling: bucket edges by (core, dst-block, src-chunk),
    pad each bucket to a multiple of 128 slots (shared across cores), and
    build per-core gather-index / selector-metadata arrays."""
    src = edge_src.astype(np.int64)
    dst = edge_dst.astype(np.int64)
    ew = edge_weight.astype(np.float32)

    core = dst // NPC
    ldst = dst - core * NPC
    blk = ldst // P                       # 0..NBLK-1
    doff = (ldst % P).astype(np.float32)
    chunk = src // CHUNK
    lsrc = (src - chunk * CHUNK).astype(np.int16)

    nseg = N_CORES * NBLK * NCHUNK
    seg = (core * NBLK + blk) * NCHUNK + chunk
    counts = np.bincount(seg, minlength=nseg).reshape(N_CORES, NBLK, NCHUNK)

    # static slot count per (block, chunk): max over cores, rounded to 128
    S = ((counts.max(axis=0) + P - 1) // P) * P      # [NBLK, NCHUNK]
    Ssg = S.reshape(NSG, G, NCHUNK)
    L = Ssg.sum(axis=1)                              # [NSG, NCHUNK] gather sizes
    SLOTS_G = L.sum(axis=1)                          # [NSG] slots per supergroup

    # offsets
    REG_OFF = np.zeros((NSG, NCHUNK), dtype=np.int64)    # msg-local region start
    REG_OFF[:, 1:] = np.cumsum(L, axis=1)[:, :-1]
    # SUB_OFF[b, ch]: msg-local slot offset of block b's bucket inside its sg
    sub = np.cumsum(Ssg, axis=1)                        # cumsum over blocks in sg
    SUB_OFF = np.zeros((NSG, G, NCHUNK), dtype=np.int64)
    SUB_OFF[:, 1:, :] = sub[:, :-1, :]
    SUB_OFF = SUB_OFF + REG_OFF[:, None, :]
    # IDXC_BASE[c][g]: start of (g, c)'s idx list within chunk c's idx stream
    IDXC_BASE = np.zeros((NCHUNK, NSG), dtype=np.int64)
    IDXC_BASE[:, 1:] = np.cumsum(L.T, axis=1)[:, :-1]
    CHUNK_TOT = L.sum(axis=0)                        # [NCHUNK] idxs per chunk
    FMAX = int(((CHUNK_TOT.max() + 15) // 16 + 7) // 8 * 8)
    BAT_BASE = np.zeros(NSG, dtype=np.int64)
    BAT_BASE[1:] = np.cumsum(SLOTS_G // P)[:-1]
    NBAT = int((SLOTS_G // P).sum())

    # rank of each edge within its (core, blk, chunk) bucket
    order = np.argsort(seg, kind="stable")
    counts_flat = counts.reshape(-1)
    starts = np.zeros(nseg, dtype=np.int64)
    starts[1:] = np.cumsum(counts_flat)[:-1]
    rank_sorted = np.arange(E_EDGES, dtype=np.int64) - starts[seg[order]]
    rank = np.empty(E_EDGES, dtype=np.int64)
    rank[order] = rank_sorted

    g_of = blk // G
    # position inside the (g, chunk) gather idx list
    within = (SUB_OFF[g_of, blk % G, chunk] - REG_OFF[g_of, chunk]) + rank
    idxpos = IDXC_BASE[chunk, g_of] + within
    # msg-local slot inside the supergroup
    slot = SUB_OFF[g_of, blk % G, chunk] + rank
    bat = BAT_BASE[g_of] + slot // P
    part = slot % P

    # per-(core, chunk) idx streams, wrapped in 16 partitions, placed in the
    # 32-partition group [32c, 32c+32) that queue c's Q7 cpu pair reads
    idx_pack = np.zeros((N_CORES, P, FMAX), dtype=np.int16)
    idx_flat = np.zeros((N_CORES, NCHUNK, FMAX * 16), dtype=np.int16)
    idx_flat[core, chunk, idxpos] = lsrc
    for c in range(NCHUNK):
        w = idx_flat[:, c, :].reshape(N_CORES, FMAX, 16).transpose(0, 2, 1)
        idx_pack[:, 32 * c:32 * c + 16, :] = w
        idx_pack[:, 32 * c + 16:32 * c + 32, :] = w

    doff_arr = np.zeros((N_CORES, NBAT, P), dtype=np.float32)
    ew_arr = np.zeros((N_CORES, NBAT, P), dtype=np.float32)
    doff_arr[core, bat, part] = doff
    ew_arr[core, bat, part] = ew
    doff_cols = np.ascontiguousarray(
        doff_arr.transpose(0, 2, 1)).astype(np.float16)   # [cores, 128, NBAT]
    ew_cols = np.ascontiguousarray(
        ew_arr.transpose(0, 2, 1)).astype(np.float16)

    meta = {
        "S": S, "L": L, "SLOTS_G": SLOTS_G, "REG_OFF": REG_OFF,
        "SUB_OFF": SUB_OFF, "IDXC_BASE": IDXC_BASE, "BAT_BASE": BAT_BASE,
        "FMAX": FMAX, "NBAT": NBAT,
    }
    return meta, idx_pack, doff_cols, ew_cols


def _build(meta):
    S = meta["S"]; L = meta["L"]; SLOTS_G = meta["SLOTS_G"]
    REG_OFF = meta["REG_OFF"]; SUB_OFF = meta["SUB_OFF"]
    IDXC_BASE = meta["IDXC_BASE"]; BAT_BASE = meta["BAT_BASE"]
    FMAX = meta["FMAX"]; NBAT = meta["NBAT"]
    NPAD = NBLK * P

    nc = bacc.Bacc("TRN2", target_bir_lowering=False, debug=False,
                   num_devices=N_CORES, num_swdge_queues=4)
    xT_in = nc.dram_tensor("xT_in", [P, NPAD], mybir.dt.float16,
                           kind="ExternalInput")
    w1_in = nc.dram_tensor("w1_in", [P, D], mybir.dt.float16, kind="ExternalInput")
    w2_in = nc.dram_tensor("w2_in", [P, D], mybir.dt.float16, kind="ExternalInput")
    idx_in = nc.dram_tensor("idx_in", [P, FMAX], mybir.dt.int16,
                            kind="ExternalInput")
    iota_in = nc.dram_tensor("iota_in", [P, P], mybir.dt.float16,
                             kind="ExternalInput")
    warm_in = nc.dram_tensor("warm_in", [P, 8], mybir.dt.int16,
                             kind="ExternalInput")
    doff_in = nc.dram_tensor("doff_in", [P, NBAT], mybir.dt.float16,
                             kind="ExternalInput")
    ew_in = nc.dram_tensor("ew_in", [P, NBAT], mybir.dt.float16,
                           kind="ExternalInput")
    out_dram = nc.dram_tensor("out", [NPC, D], mybir.dt.float32,
                              kind="ExternalOutput")

    with tile.TileContext(nc) as tc:
        with tc.tile_pool(name="const", bufs=1) as const, \
             tc.tile_pool(name="xt", bufs=1) as xtp, \
             tc.tile_pool(name="msg", bufs=3) as msgp, \
             tc.tile_pool(name="sel", bufs=3) as selp, \
             tc.tile_pool(name="hcast", bufs=4) as hcp, \
             tc.tile_pool(name="outp", bufs=4) as outp, \
             tc.tile_pool(name="psA", bufs=4, space="PSUM") as psA, \
             tc.tile_pool(name="psD", bufs=2, space="PSUM") as psD, \
             tc.tile_pool(name="dram", bufs=1, space="DRAM") as dram:

            # ---- resident constants ----
            iota_i32 = const.tile([P, P], mybir.dt.int32)
            nc.gpsimd.iota(iota_i32[:], pattern=[[1, P]], base=0,
                           channel_multiplier=0)
            iota_bf = const.tile([P, P], mybir.dt.float16)
            nc.vector.tensor_copy(out=iota_bf[:], in_=iota_i32[:])

            w1_t = const.tile([P, D], mybir.dt.float16)
            nc.sync.dma_start(out=w1_t[:], in_=w1_in[:])
            w2_t = const.tile([P, D], mybir.dt.float16)
            nc.sync.dma_start(out=w2_t[:], in_=w2_in[:])
            idx_t = const.tile([P, FMAX], mybir.dt.int16)
            nc.sync.dma_start(out=idx_t[:], in_=idx_in[:])
            doff_t = const.tile([P, NBAT], mybir.dt.float16)
            nc.sync.dma_start(out=doff_t[:], in_=doff_in[:])
            ew_t = const.tile([P, NBAT], mybir.dt.float16)
            nc.sync.dma_start(out=ew_t[:], in_=ew_in[:])
            x1T = xtp.tile([P, NPAD], mybir.dt.float16, tag="xT")
            nc.sync.dma_start(out=x1T[:], in_=xT_in[:])

            # ---- DRAM internals ----
            h_bounce = [dram.tile([NPC, D], mybir.dt.float16, tag=f"hb{i}",
                                  name=f"h_bounce{i}") for i in range(2)]
            h_full = [dram.tile([N_NODES, D], mybir.dt.float16, tag=f"hf{i}",
                                addr_space=("Shared" if USE_SHARED else "Local"),
                                name=f"h_full{i}")
                      for i in range(2)]

            def dense(xT_tile, w_tile, bounce):
                for t in range(NBLK):
                    ps = psD.tile([P, D], mybir.dt.float32, tag="dense")
                    nc.tensor.matmul(out=ps[:], lhsT=xT_tile[:, t * P:(t + 1) * P],
                                     rhs=w_tile[:], start=True, stop=True)
                    hb = hcp.tile([P, D], mybir.dt.float16, tag="hcast")
                    nc.scalar.activation(out=hb[:], in_=ps[:],
                                         func=mybir.ActivationFunctionType.Copy)
                    rows = min(P, NPC - t * P)
                    nc.sync.dma_start(out=bounce[t * P:t * P + rows, :],
                                      in_=hb[:rows, :])

            def allgather(bounce, full):
                nc.gpsimd.collective_compute(
                    "AllGather", mybir.AluOpType.bypass,
                    replica_groups=[list(range(N_CORES))],
                    ins=[bounce.opt()], outs=[full.opt()],
                )

            def emit_gathers(msg, full, g, sems=None):
                for ch in range(NCHUNK):
                    lg = int(L[g, ch])
                    if lg == 0:
                        continue
                    r0 = int(REG_OFF[g, ch]) // P
                    i0 = int(IDXC_BASE[ch, g]) // 16
                    kw = dict(
                        out_ap=msg[:, r0:r0 + lg // P, :],
                        in_ap=full[ch * CHUNK:(ch + 1) * CHUNK, :],
                        idxs_ap=idx_t[:, i0:i0 + lg // 16],
                        num_idxs=lg, num_idxs_reg=lg, elem_size=D,
                        queue_num=ch, single_packet=False,
                    )
                    if sems is not None:
                        nc.gpsimd.dma_gather(prepare_only=True, sem=sems[ch],
                                             **kw)
                    else:
                        nc.gpsimd.dma_gather(**kw)

            def prefetch(full, layer):
                """Emit descriptor-gen for the first PREF supergroups right
                after the AllGather: desc-gen has no data dep on the table
                (deferred to trigger_dma), so the Q7 pairs build descriptors
                while the AG is in flight and the DMAs fire the moment it
                lands. Tile's DMASW-lane bookkeeping under-synchronizes
                consumers of prepared DMAs (later in-order DMAs on the lane
                satisfy the count early), so consumers are gated manually via
                dma_sems + a Tensor-engine wait_ge in aggregate()."""
                sems = [nc.alloc_semaphore(f"prep{layer}_{ch}")
                        for ch in range(NCHUNK)]
                n_preps = [int((L[:PREF, ch] > 0).sum()) for ch in range(NCHUNK)]
                pre = {}
                for g in range(PREF):
                    nb = int(SLOTS_G[g]) // P
                    msg = msgp.tile([P, nb, D], mybir.dt.float16, tag="msg")
                    emit_gathers(msg, full, g, sems=sems)
                    pre[g] = msg
                return pre, (sems, n_preps)

            def aggregate(full, layer, x2T, pre, gate):
                sems, n_preps = gate
                if pre:
                    for ch in range(NCHUNK):
                        nc.gpsimd.trigger_dma(count=None, queue_num=ch)
                for g in range(NSG):
                    nslot = int(SLOTS_G[g])
                    nb = nslot // P
                    gb0 = int(BAT_BASE[g])
                    if g in pre:
                        msg = pre[g]
                    else:
                        msg = msgp.tile([P, nb, D], mybir.dt.float16, tag="msg")
                        emit_gathers(msg, full, g)
                    # wide selector build: sel[e, b, d] = ew[e,b]*(d==doff[e,b])
                    wsel = selp.tile([P, nb, P], mybir.dt.float16, tag="sel")
                    iota_b = iota_bf[:].unsqueeze(1).broadcast_to([P, nb, P])
                    doff_b = doff_t[:, gb0:gb0 + nb].unsqueeze(2) \
                        .broadcast_to([P, nb, P])
                    ew_b = ew_t[:, gb0:gb0 + nb].unsqueeze(2) \
                        .broadcast_to([P, nb, P])
                    if g in pre:
                        # Build the one-hot per chunk region, gating each on
                        # its queue's prepared-DMA completion semaphore; the
                        # agg matmuls read wsel so the wait propagates via
                        # real RAW deps (Tile's DMASW bookkeeping under-syncs
                        # consumers of prepared DMAs).
                        for ch in range(NCHUNK):
                            lgc = int(L[g, ch])
                            if lgc == 0:
                                continue
                            r0 = int(REG_OFF[g, ch]) // P
                            nbc = lgc // P
                            eq = nc.vector.tensor_tensor(
                                out=wsel[:, r0:r0 + nbc, :],
                                in0=iota_bf[:].unsqueeze(1)
                                    .broadcast_to([P, nbc, P]),
                                in1=doff_t[:, gb0 + r0:gb0 + r0 + nbc]
                                    .unsqueeze(2).broadcast_to([P, nbc, P]),
                                op=mybir.AluOpType.is_equal)
                            eq.wait_op(sems[ch], 16 * n_preps[ch], "sem-ge")
                    else:
                        nc.vector.tensor_tensor(out=wsel[:], in0=iota_b,
                                                in1=doff_b,
                                                op=mybir.AluOpType.is_equal)
                    nc.vector.tensor_tensor(out=wsel[:], in0=wsel[:], in1=ew_b,
                                            op=mybir.AluOpType.mult)
                    for j in range(G):
                        b = g * G + j
                        batches = []
                        for ch in range(NCHUNK):
                            nbj = int(S[b, ch]) // P
                            s0 = int(SUB_OFF[g, j, ch]) // P
                            batches += [s0 + k for k in range(nbj)]
                        rows = min(P, NPC - b * P)
                        ps = psA.tile([P, D], mybir.dt.float32, tag="agg")
                        for i, s in enumerate(batches):
                            first, last = (i == 0), (i == len(batches) - 1)
                            if layer == 1:
                                # psum[feat, dst] = sum_e msg[e, f] sel[e, d]
                                nc.tensor.matmul(out=ps[:], lhsT=msg[:, s, :],
                                                 rhs=wsel[:, s, :], start=first,
                                                 stop=last)
                            else:
                                # psum[dst, feat]
                                nc.tensor.matmul(out=ps[:], lhsT=wsel[:, s, :],
                                                 rhs=msg[:, s, :], start=first,
                                                 stop=last)
                        if layer == 1:
                            nc.scalar.activation(
                                out=x2T[:, b * P:(b + 1) * P], in_=ps[:],
                                func=mybir.ActivationFunctionType.Tanh)
                        else:
                            ot = outp.tile([P, D], mybir.dt.float32, tag="out")
                            nc.scalar.activation(
                                out=ot[:], in_=ps[:],
                                func=mybir.ActivationFunctionType.Tanh)
                            nc.sync.dma_start(out=out_dram[b * P:b * P + rows, :],
                                              in_=ot[:rows, :])

            dense(x1T, w1_t, h_bounce[0])
            allgather(h_bounce[0], h_full[0])
            pre1, gate1 = prefetch(h_full[0], 1)
            x2T = xtp.tile([P, NPAD], mybir.dt.float16, tag="xT")
            aggregate(h_full[0], 1, x2T, pre1, gate1)
            dense(x2T, w2_t, h_bounce[1])
            allgather(h_bounce[1], h_full[1])
            pre2, gate2 = prefetch(h_full[1], 2)
            aggregate(h_full[1], 2, None, pre2, gate2)

    nc.compile()
    return nc


def kernel(x, edge_src, edge_dst, edge_weight, W1, W2, _trace=False):
    assert x.shape == (N_NODES, D) and edge_src.shape == (E_EDGES,)
    meta, idx_pack, doff_c, ew_c = _prep(x, edge_src, edge_dst, edge_weight)
    nc = _build(meta)

    NPAD = NBLK * P
    w1 = np.ascontiguousarray(W1.astype(BF16))
    w2 = np.ascontiguousarray(W2.astype(BF16))
    in_maps = []
    for c in range(N_CORES):
        xT = np.zeros((P, NPAD), dtype=BF16)
        xT[:, :NPC] = x[c * NPC:(c + 1) * NPC].T.astype(BF16)
        in_maps.append({
            "xT_in": xT,
            "w1_in": w1, "w2_in": w2,
            "idx_in": idx_pack[c],
            "doff_in": doff_c[c],
            "ew_in": ew_c[c],
        })
    res = run_bass_kernel_spmd(nc, in_maps, core_ids=list(range(N_CORES)),
                               trace=_trace)
    out = np.concatenate([res.results[c]["out"] for c in range(N_CORES)], axis=0)
    if _trace:
        kernel.last_results = res
    return out
